# revision 64
# baseline (speedup 1.0000x reference)
"""Trainium2 Bass kernel for nn_MultiAttention (GQA+MLA attention, gated, SwiGLU out).

Sharding (8 cores, core c = b*4 + g):
- Attention: data-parallel over batch b, tensor-parallel over KV head g
  (4 q-heads + 1 kv head per core), all 2048 queries, causal structure
  identical on every core (single SPMD program).
- Reshard: FOUR per-chunk 8-core AllToAlls; each destination selects its
  batch's 4 source slots with a 0/1 mask (SPMD-uniform).
- MLP: data-parallel over batch + token-class parallel: core (b, q')
  processes the 512 tokens {512j + 128 q' + i}, full DFF, with wo1/wo2
  resident in SBUF as bf16 (preloaded during attention).

Attention inner loop processes q-head PAIRS: the two K=64 score matmuls
(rows 0-63 / 64-127 via the host-duplicated k columns) are issued
back-to-back so the PE runs them concurrently in disjoint row-groups.
The causal-diagonal s-tiles restrict scores/exp/AV to the valid column
range (col >= 128*i); the boundary triangle is masked by a small N=128
matmul (negid @ tri) per head bank, so exp underflows to exactly 0 and
nothing else is on the softmax critical path. Softmax: no max-subtraction
(scores bounded), denominator via an appended ones-column in V (psum row
64), normalization via a K=1 broadcast matmul. PSUM budget: score pair
tiles [128,2,512] x2 (4 banks) + psO x2 (2) + misc x2 (2) = 8 banks,
with tails split into an immediate normalize (frees psO) and a deferred
gate chain. Both sigmoids (gate and SwiGLU) are computed via tanh --
which shares the ACT table set with exp, so the kernel never switches
activation tables -- with the /2 constants folded into Wo1/Wo2 on the
host. psum->sbuf copies run on DVE, keeping ACT on the exp stream (the
attention-phase bottleneck). AllToAll readback is 4 merged descriptors
per round instead of 32 singles; round-1 readback issues from the
(idle) Pool queue so nothing queues behind the collective-done
semaphore, and the resident wo1/wo2 stream in as 16 1MB slices always
queued behind the latency-critical x prefetches.
"""

import numpy as np
import ml_dtypes

import concourse.bacc as bacc
import concourse.bass as bass
import concourse.mybir as mybir
import concourse.tile as tile
from concourse.bass_utils import run_bass_kernel_spmd

# problem dims
B, T, DM = 2, 2048, 1024
H, HKV, DH = 16, 4, 64
LAT, DFF = 64, 4096
SCALE = DH ** -0.5
ROPE_BASE = 10000.0

N_CORES = 8
TCH = 512                 # t-chunk (query chunk)
HCH = 256                 # x half-chunk (DMA granularity)
NCH = T // TCH            # 4 chunks
NHL = H // HKV            # 4 local q-heads per core
QT = 512                  # MLP tokens per core
RT = 256                  # MLP tokens per round (2 rounds)
KT_DM = DM // 128         # 8 k-tiles over model dim
NKL = DFF // 128          # 32 k-tiles over ff dim
# Masked scores get NEG_BIG added pre-exp: big enough that
# exp(SCALE*(s+NEG_BIG)) underflows to exactly 0.0 in f32 for any real
# score s, small enough to stay in the activation table's sane range.
NEG_BIG = -1.0e5
f32 = mybir.dt.float32
bf16 = mybir.dt.bfloat16

_cache = {}


def _build_nc():
    nc = bacc.Bacc("TRN2", target_bir_lowering=False, debug=False,
                   num_devices=N_CORES)

    # ---- DRAM I/O (bf16 unless noted) ----
    xT = nc.dram_tensor("xT", [DM, T], bf16, kind="ExternalInput")
    wq = nc.dram_tensor("wq", [DM, NHL * DH], bf16, kind="ExternalInput")
    wkv = nc.dram_tensor("wkv", [DM, LAT], bf16, kind="ExternalInput")
    wk = nc.dram_tensor("wk", [LAT, 2 * DH], bf16, kind="ExternalInput")
    perm = nc.dram_tensor("perm", [128, 128], bf16, kind="ExternalInput")
    wv = nc.dram_tensor("wv", [LAT, DH], bf16, kind="ExternalInput")
    wg = nc.dram_tensor("wg", [DH, DH], bf16, kind="ExternalInput")
    bgn = nc.dram_tensor("bgn", [DH, 1], f32, kind="ExternalInput")  # 0.5*bgate
    wo1 = nc.dram_tensor("wo1", [DM, DFF], bf16, kind="ExternalInput")
    wo2 = nc.dram_tensor("wo2", [DFF, DM], bf16, kind="ExternalInput")
    cs2 = nc.dram_tensor("cs2", [128, T], bf16, kind="ExternalInput")
    ns2 = nc.dram_tensor("ns2", [128, T], bf16, kind="ExternalInput")
    tri = nc.dram_tensor("tri", [128, 128], bf16, kind="ExternalInput")
    negid = nc.dram_tensor("negid", [128, 128], bf16, kind="ExternalInput")
    ones65 = nc.dram_tensor("ones65", [65, DH], bf16, kind="ExternalInput")
    selA = nc.dram_tensor("selA", [128, 1], f32, kind="ExternalInput")
    selB = nc.dram_tensor("selB", [128, 1], f32, kind="ExternalInput")
    yT = nc.dram_tensor("yT", [DM, QT], bf16, kind="ExternalOutput")

    with tile.TileContext(nc) as tc:
        _body(nc, tc, xT, wq, wkv, wk, wv, wg, bgn, wo1, wo2, cs2, ns2,
              tri, negid, ones65, selA, selB, yT, perm)
    nc.compile()
    return nc


def _body(nc, tc, xT, wq, wkv, wk, wv, wg, bgn, wo1, wo2, cs2, ns2,
          tri, negid, ones65, selA, selB, yT, perm):
    Exp = mybir.ActivationFunctionType.Exp
    Tanh = mybir.ActivationFunctionType.Tanh
    Copy = mybir.ActivationFunctionType.Copy

    with (
        tc.tile_pool(name="const", bufs=1) as const,
        tc.tile_pool(name="wres", bufs=1) as wres,
        tc.tile_pool(name="qk", bufs=1) as qk,
        tc.tile_pool(name="ppool", bufs=1) as ppool,
        tc.tile_pool(name="tmp", bufs=1) as tmp,
        tc.tile_pool(name="dram", bufs=1, space="DRAM") as dram,
    ):
        # ---- constants / small weights (live whole kernel) ----
        # first-k-tile slices of x and wq land first so the very first
        # qproj matmul isn't gated on the full 1MB of startup DMA
        x00 = qk.tile([128, KT_DM, HCH], bf16, tag="x", bufs=2,
                      name="x_0_0")
        xT_r0 = xT[:].rearrange("(kt p) t -> p kt t", p=128)
        wq_t = const.tile([128, KT_DM, NHL * DH], bf16)
        wq_ap = wq[:].rearrange("(kt p) m -> p kt m", p=128)
        nc.sync.dma_start(out=x00[:, 0:1, :], in_=xT_r0[:, 0:1, 0:HCH])
        nc.sync.dma_start(out=wq_t[:, 0:1, :], in_=wq_ap[:, 0:1, :])
        nc.sync.dma_start(out=x00[:, 1:KT_DM, :],
                          in_=xT_r0[:, 1:KT_DM, 0:HCH])
        nc.sync.dma_start(out=wq_t[:, 1:KT_DM, :], in_=wq_ap[:, 1:KT_DM, :])
        # x chunk-0 second half next: the first qproj needs it right away
        x01 = qk.tile([128, KT_DM, HCH], bf16, tag="x", bufs=2,
                      name="x_0_1")
        nc.sync.dma_start(out=x01[:], in_=xT_r0[:, :, HCH:TCH])
        # then in order of first use: latk weights, rope tables (first rope
        # is ~6us in), mask/misc tables, big wo loads come per-chunk later
        wkv_t = const.tile([128, KT_DM, LAT], bf16)
        nc.sync.dma_start(out=wkv_t[:],
                          in_=wkv[:].rearrange("(kt p) m -> p kt m", p=128))
        wk_t = const.tile([LAT, 2 * DH], bf16)
        perm_t = const.tile([128, 128], bf16)
        nc.sync.dma_start(out=wk_t[:], in_=wk[:, :])
        nc.sync.dma_start(out=perm_t[:], in_=perm[:, :])
        cs_t = const.tile([128, T], bf16)
        ns_t = const.tile([128, T], bf16)
        nc.sync.dma_start(out=cs_t[:, 0:TCH], in_=cs2[:, 0:TCH])
        nc.sync.dma_start(out=ns_t[:, 0:TCH], in_=ns2[:, 0:TCH])
        tri_t = const.tile([128, 128], bf16)
        nc.sync.dma_start(out=tri_t[:], in_=tri[:, :])
        negid_t = const.tile([128, 128], bf16)
        nc.sync.dma_start(out=negid_t[:], in_=negid[:, :])
        wv_t = const.tile([LAT, DH], bf16)
        wg_t = const.tile([DH, DH], bf16)
        nc.sync.dma_start(out=wv_t[:], in_=wv[:, :])
        nc.sync.dma_start(out=wg_t[:], in_=wg[:, :])
        ones_t = const.tile([65, DH], bf16)
        nc.sync.dma_start(out=ones_t[:], in_=ones65[:, :])
        selA_t = const.tile([128, 1], f32)
        selB_t = const.tile([128, 1], f32)
        nc.sync.dma_start(out=selA_t[:], in_=selA[:, :])
        nc.sync.dma_start(out=selB_t[:], in_=selB[:, :])
        bgn_t = const.tile([DH, 1], f32)
        nc.sync.dma_start(out=bgn_t[:], in_=bgn[:, :])
        nc.sync.dma_start(out=cs_t[:, TCH:T], in_=cs2[:, TCH:T])
        nc.sync.dma_start(out=ns_t[:, TCH:T], in_=ns2[:, TCH:T])

        # resident MLP weights; the DMAs are interleaved into the chunk loop
        wo1_t = wres.tile([128, KT_DM, DFF], bf16)
        wo2_t = wres.tile([128, NKL, DM], bf16)
        wo1_ap = wo1[:].rearrange("(kt p) f -> p kt f", p=128)
        wo2_ap = wo2[:].rearrange("(kl p) m -> p kl m", p=128)

        # reshard buffers: slot d = destination core, rows = 4 local
        # heads x 64, cols = one 128-token destination class
        cin = [dram.tile([8, NHL * DH, 128], bf16, name=f"cin{j}")
               for j in range(NCH)]
        cout = [dram.tile([8, NHL * DH, 128], bf16, name=f"cout{j}")
                for j in range(NCH)]

        xT_r = xT[:].rearrange("(kt p) t -> p kt t", p=128)

        # round-r gated-attention activations, cols = (kt, jr, 128)
        attFs = [tmp.tile([128, KT_DM, 2, 128], bf16, tag="attF", bufs=2,
                          name=f"attF{r}") for r in range(2)]

        def readback(r, jr, eng):
            """DMA cout[2r+jr]'s two batch halves into staging, select."""
            src = cout[2 * r + jr][:].rearrange("s (hh p) x -> p s hh x",
                                                p=128)
            bAt = tmp.tile([128, 4, 2, 128], bf16, tag="blk", bufs=2,
                           name=f"bA{r}{jr}")
            bBt = tmp.tile([128, 4, 2, 128], bf16, tag="blk", bufs=2,
                           name=f"bB{r}{jr}")
            eng.dma_start(out=bAt[:], in_=src[:, 0:4])
            eng.dma_start(out=bBt[:], in_=src[:, 4:8])
            return bAt, bBt

        def rb_select(r, jr, bAt, bBt, veng):
            # attF cols are (kt, jr, x) with kt = (s, hh) row-major, so the
            # fixed-jr slice lines up elementwise with the staging tiles
            af = attFs[r][:, :, jr, :]
            with nc.allow_low_precision(reason="0/1 batch select"):
                veng.tensor_scalar_mul(bAt[:], bAt[:], selA_t[:])
                veng.tensor_scalar_mul(bBt[:], bBt[:], selB_t[:])
                veng.tensor_add(
                    af, bAt[:].rearrange("p s hh x -> p (s hh) x"),
                    bBt[:].rearrange("p s hh x -> p (s hh) x"))

        def x_dma(jc, th):
            t = qk.tile([128, KT_DM, HCH], bf16, tag="x", bufs=2,
                        name=f"x_{jc}_{th}")
            lo = jc * TCH + th * HCH
            nc.sync.dma_start(out=t[:], in_=xT_r[:, :, lo:lo + HCH])
            return t

        # ================= attention =================
        with tc.tile_pool(name="psA", bufs=1, space="PSUM") as psum:

            k_pre = [qk.tile([128, TCH], bf16, tag="kp", bufs=NCH,
                             name=f"k_pre{jc}") for jc in range(NCH)]
            v_sb = [qk.tile([128, 4, DH + 1], bf16, tag="vsb", bufs=NCH,
                            name=f"v_sb{jc}") for jc in range(NCH)]

            def rope_sb(pre, jc):
                # Split-half rotary in place on `pre` [128, TCH]. The
                # half-swap is a pair of 64-row permutation matmuls (the
                # perm matrix is block-diagonal, so the two halves run in
                # disjoint PE row/col groups); DVE combines
                # pre*cos + swapped*sin.
                psw = psum.tile([128, TCH], f32, tag="psm", bufs=2,
                                name="psw")
                nc.tensor.matmul(psw[0:64, :], perm_t[0:64, 0:64],
                                 pre[0:64, :], start=True, stop=True)
                nc.tensor.matmul(psw[64:128, :], perm_t[64:128, 64:128],
                                 pre[64:128, :], start=True, stop=True)
                # the staging tag is idle during attention; borrow a slot
                sw = tmp.tile([128, TCH], bf16, tag="blk", bufs=2,
                              name="swap")
                cs = cs_t[:, jc * TCH:(jc + 1) * TCH]
                ns = ns_t[:, jc * TCH:(jc + 1) * TCH]
                with nc.allow_low_precision(reason="bf16 rope"):
                    # pre*cos first: independent of the psw matmul, so it
                    # overlaps the PE swap instead of extending the chain
                    nc.vector.tensor_mul(pre[:], pre[:], cs)
                    nc.vector.tensor_mul(sw[:], psw[:], ns)
                    nc.vector.tensor_add(pre[:], pre[:], sw[:])

            # deferred AV emission: AV pair of s-tile t is emitted after the
            # scores+exp of tile t+2, so PE always has independent score
            # tiles in flight while ACT catches up.
            pend_av = []

            def emit_av(ent):
                psOs, t, pt, lo, first, last = ent
                for ab in range(2):
                    nc.tensor.matmul(
                        psOs[ab][:, lo:TCH], v_sb[t // 4][:, t % 4, :],
                        pt[:, ab, lo:TCH],
                        start=(first and lo == 0), stop=last,
                    )

            def flush_av(depth=0):
                while len(pend_av) > depth:
                    emit_av(pend_av.pop(0))

            def normalize(st):
                # immediate tail part: softmax denominators -> oT, frees psO
                psOs, oTs = st["psO"], st["oT"]
                # any of this pair's AVs still deferred must land first
                while pend_av and pend_av[0][0] is psOs:
                    emit_av(pend_av.pop(0))
                for ab in range(2):
                    rden = tmp.tile([65, TCH], bf16, tag="rden", bufs=2,
                                    name=f"rden{ab}")
                    with nc.allow_low_precision(reason="denom recip"):
                        nc.vector.reciprocal(rden[64:65, :],
                                             psOs[ab][64:65, :])
                    # ones row at partition 64: K=1 matmul broadcasts the
                    # reciprocal row to 64 partitions (base 0, so the DVE
                    # normalize keeps all operands partition-aligned)
                    psD = psum.tile([64, TCH], f32, tag="psm", bufs=2,
                                    name="psD")
                    nc.tensor.matmul(psD[:], ones_t[64:65, :],
                                     rden[64:65, :], start=True, stop=True)
                    dfac = tmp.tile([64, TCH], bf16, tag="dfac", bufs=2,
                                    name="dfac")
                    nc.scalar.activation(dfac[:], psD[:], Copy)
                    with nc.allow_low_precision(reason="softmax normalize"):
                        nc.vector.tensor_mul(oTs[ab][:], psOs[ab][0:64, :],
                                             dfac[:])

            def gate_tail(st):
                # deferred tail part: sigmoid gate + reshard write
                jc, oTs = st["jc"], st["oT"]
                for ab in range(2):
                    h = 2 * st["hp"] + ab
                    oT = oTs[ab]
                    psG = psum.tile([64, TCH], f32, tag="psm", bufs=2,
                                    name="psG")
                    nc.tensor.matmul(psG[:], wg_t[:], oT[:],
                                     start=True, stop=True)
                    # sigmoid(z) = (tanh(z/2)+1)/2: tanh shares the ACT
                    # table set with exp (no table switch anywhere in the
                    # kernel); the /2 is folded into Wo1 host-side, so the
                    # gated output written out is oT*(tanh+1) = 2*gated
                    th = tmp.tile([64, TCH], bf16, tag="eg", bufs=1,
                                  name="th")
                    nc.scalar.activation(th[:], psG[:], Tanh, bias=bgn_t[:],
                                         scale=0.5)
                    with nc.allow_low_precision(reason="sigmoid via tanh"):
                        nc.vector.tensor_mul(th[:], oT[:], th[:])
                        nc.vector.tensor_add(oT[:], oT[:], th[:])
                    # SBUF source keeps the partition dim first; the class
                    # split lives in the DRAM-side access pattern.
                    cin_v = cin[jc][:].rearrange(
                        "(b c) (hh p) x -> b hh p c x", b=2, p=64)
                    oT_v = oT[:].rearrange("p (c x) -> p c x", c=4)
                    for bb in range(2):
                        nc.sync.dma_start(out=cin_v[bb, h], in_=oT_v)

            def alltoall(cj):
                nc.gpsimd.collective_compute(
                    "AllToAll", mybir.AluOpType.bypass,
                    replica_groups=[list(range(8))],
                    ins=[cin[cj][:].opt()],
                    outs=[cout[cj][:].opt()],
                )

            q2s = [None] * NCH

            def make_prep(jc):
                """Chunk-jc projections as 4 emission pieces, interleaved
                into the previous chunk's score stream as PE filler."""
                st = {}

                # chunk 0 prep runs before any scores: borrow the (idle)
                # psS region so projections overlap instead of serializing
                # on the 2-slot misc rotation
                big = "psS" if jc == 0 else "psm"

                def qproj(m):
                    if m == 0:
                        if jc == 0:
                            st["x"] = [x00, x01]
                        else:
                            st["x"] = [x_dma(jc, 0), x_dma(jc, 1)]
                        st["q2"] = qk.tile([128, 2, TCH], bf16, tag="q2",
                                           bufs=2, name=f"q2_{jc}")
                        q2s[jc] = st["q2"]
                    ps = psum.tile([128, TCH], f32, tag=big, bufs=2,
                                   name="psq")
                    for th in range(2):
                        for kt in range(KT_DM):
                            nc.tensor.matmul(
                                ps[:, th * HCH:(th + 1) * HCH],
                                wq_t[:, kt, m * 128:(m + 1) * 128],
                                st["x"][th][:, kt, :],
                                start=(kt == 0), stop=(kt == KT_DM - 1),
                            )
                    nc.vector.tensor_copy(st["q2"][:, m, :], ps[:])
                    rope_sb(st["q2"][:, m, :], jc)

                def latk():
                    lat = tmp.tile([LAT, TCH], bf16, tag="lat", bufs=1,
                                   name=f"lat_{jc}")
                    st["lat"] = lat
                    ps = psum.tile([64, TCH], f32, tag=big, bufs=2,
                                   name="pslat")
                    for th in range(2):
                        for kt in range(KT_DM):
                            nc.tensor.matmul(
                                ps[:, th * HCH:(th + 1) * HCH],
                                wkv_t[:, kt, :], st["x"][th][:, kt, :],
                                start=(kt == 0), stop=(kt == KT_DM - 1))
                    nc.vector.tensor_copy(lat[:], ps[:])
                    ps = psum.tile([128, TCH], f32, tag="psm", bufs=2,
                                   name="psk")
                    # wk has host-duplicated columns: k projects straight to
                    # 128 rows (two stacked copies), no dup DMA needed
                    nc.tensor.matmul(ps[:], wk_t[:], lat[:],
                                     start=True, stop=True)
                    nc.vector.tensor_copy(k_pre[jc][:], ps[:])
                    rope_sb(k_pre[jc][:], jc)

                def vproj():
                    nc.gpsimd.memset(v_sb[jc][:, :, DH:DH + 1], 1.0)
                    for tl in range(4):
                        ps = psum.tile([128, DH], f32, tag="psm", bufs=2,
                                       name="psv")
                        nc.tensor.matmul(
                            ps[:], st["lat"][:, tl * 128:(tl + 1) * 128],
                            wv_t[:], start=True, stop=True)
                        nc.vector.tensor_copy(v_sb[jc][:, tl, 0:DH], ps[:])

                return [lambda: qproj(0), latk, lambda: qproj(1), vproj]

            # chunk 0 prep runs cold; emit PE work densely, engines overlap
            prep0 = make_prep(0)
            for p in prep0:
                p()
            prev_norm = [None]   # pair state awaiting normalize
            prev_gate = [None]   # pair state awaiting gate tail
            rb0 = [None]

            # resident MLP weights stream in as 16 x 1MB slices at fixed
            # points in the score loop, always queued BEHIND the x
            # prefetches so they never starve the latency-critical DMAs
            wo_slices = ([("wo1", kt) for kt in range(KT_DM)] +
                         [("wo2", s) for s in range(8)])

            def wo_feed():
                if not wo_slices:
                    return
                which, s = wo_slices.pop(0)
                if which == "wo1":
                    nc.sync.dma_start(out=wo1_t[:, s, :],
                                      in_=wo1_ap[:, s, :])
                else:
                    nc.sync.dma_start(out=wo2_t[:, 4 * s:4 * s + 4, :],
                                      in_=wo2_ap[:, 4 * s:4 * s + 4, :])

            # one shared silu buffer: round 1's writes naturally wait for
            # round 0's mlp2 reads (rounds are sequential), saving 16KB
            hg = tmp.tile([128, NKL, RT], bf16, tag="hT", bufs=1, name="hg")
            hgs = [hg, hg]

            def mlp1_mm(r, mt2, ptag="psS"):
                psH = psum.tile([128, 2, RT], f32, tag=ptag, bufs=2,
                                name="psH")
                for i in range(2):
                    mt = 2 * mt2 + i
                    for kt in range(KT_DM):
                        nc.tensor.matmul(
                            psH[:, i, :],
                            wo1_t[:, kt, mt * 128:(mt + 1) * 128],
                            attFs[r][:, kt, :, :].rearrange(
                                "p a x -> p (a x)"),
                            start=(kt == 0), stop=(kt == KT_DM - 1),
                        )
                return psH

            def mlp1_sig(r, mt2, psH):
                hs = hgs[r][:, 2 * mt2:2 * mt2 + 2, :] \
                    .rearrange("p a t -> p (a t)")
                psf = psH[:].rearrange("p a t -> p (a t)")
                # silu(x) = x*sigmoid(x) = x*(tanh(x/2)+1)/2: tanh keeps the
                # exp table set resident; the /2 is folded into Wo2
                # host-side, so hgs holds 2*silu
                nc.scalar.activation(hs, psf, Tanh, scale=0.5)
                with nc.allow_low_precision(reason="bf16 silu"):
                    # hs = psf*(1+tanh): single psum read frees psH earlier
                    nc.vector.tensor_scalar_add(hs, hs, 1.0)
                    nc.vector.tensor_mul(hs, psf, hs)

            def mlp1_piece(r, lo, hi):
                for mt2 in range(lo, hi):
                    mlp1_sig(r, mt2, mlp1_mm(r, mt2))

            for jc in range(NCH):
                q2 = q2s[jc]
                nxt = make_prep(jc + 1) if jc + 1 < NCH else []
                n_t = 4 * (jc + 1)
                for hp in range(2):
                    pieces = [nxt[2 * hp], nxt[2 * hp + 1]] if nxt else []
                    st = {
                        "jc": jc, "hp": hp,
                        "psO": [psum.tile([65, TCH], f32, tag="psO",
                                          bufs=2, name=f"psO{ab}")
                                for ab in range(2)],
                        "oT": [qk.tile([64, TCH], bf16, tag="oT", bufs=4,
                                       name=f"oT{ab}") for ab in range(2)],
                    }
                    for t in range(n_t):
                        if t == 1 and prev_norm[0] is not None:
                            normalize(prev_norm[0])
                            prev_gate[0] = prev_norm[0]
                            prev_norm[0] = None
                        if t == min(2, n_t - 1) and prev_gate[0] is not None:
                            g = prev_gate[0]
                            prev_gate[0] = None
                            gate_tail(g)
                            if g["jc"] != jc and g["hp"] == 1:
                                # that was chunk jc-1's last tail
                                alltoall(g["jc"])
                        if t == 1 and len(pieces) > 0:
                            pieces[0]()
                        if t == n_t // 2 + 1 and len(pieces) > 1:
                            pieces[1]()
                        if t == 2 and len(pieces) > 2:
                            pieces[2]()
                        if t in (3, 7, 11, 14):
                            wo_feed()
                        if jc == 3 and t == 2:
                            if hp == 0:
                                rb0[0] = readback(0, 0, nc.sync)
                            else:
                                rb0j1 = readback(0, 1, nc.sync)
                        if jc == 3 and t == 6:
                            if hp == 0:
                                rb_select(0, 0, *rb0[0], nc.gpsimd)
                            else:
                                rb_select(0, 1, *rb0j1, nc.gpsimd)

                        i = t - 4 * jc
                        diag = i >= 0
                        lo = 128 * i if i > 0 else 0
                        psS = psum.tile([128, 2, TCH], f32, tag="psS",
                                        bufs=2, name="psS")
                        for ab in range(2):
                            nc.tensor.matmul(
                                psS[:, ab, lo:TCH],
                                k_pre[t // 4][64 * ab:64 * ab + 64,
                                              (t % 4) * 128:
                                              (t % 4 + 1) * 128],
                                q2[64 * ab:64 * ab + 64, hp, lo:TCH],
                                start=True, stop=not diag,
                            )
                        if diag:
                            for ab in range(2):
                                nc.tensor.matmul(
                                    psS[:, ab, lo:lo + 128], negid_t[:],
                                    tri_t[:], start=False, stop=True,
                                )
                        pt = ppool.tile([128, 2, TCH], bf16, tag="P",
                                        bufs=4, name="P")
                        nc.scalar.activation(
                            pt[:, :, lo:TCH], psS[:, :, lo:TCH],
                            Exp, scale=SCALE,
                        )
                        flush_av(depth=2)
                        pend_av.append((st["psO"], t, pt, lo,
                                        t == 0, t == n_t - 1))
                    prev_norm[0] = st

            # ============ MLP (2 rounds of RT tokens) ============
            flush_av()
            normalize(prev_norm[0])
            yT_r = yT[:].rearrange("(kt p) t -> p kt t", p=128)

            def mlp2(r):
                for dmt in range(KT_DM):
                    psY = psum.tile([128, RT], f32, tag="psO", bufs=2,
                                    name="psY")
                    for kl in range(NKL):
                        nc.tensor.matmul(
                            psY[:], wo2_t[:, kl, dmt * 128:(dmt + 1) * 128],
                            hgs[r][:, kl, :],
                            start=(kl == 0), stop=(kl == NKL - 1),
                        )
                    y_sb = tmp.tile([128, RT], bf16, tag="ysb", bufs=1,
                                    name="y_sb")
                    nc.scalar.activation(y_sb[:], psY[:], Copy)
                    nc.sync.dma_start(out=yT_r[:, dmt, r * RT:(r + 1) * RT],
                                      in_=y_sb[:])

            # MM-only MLP1 pieces fill PE while the last normalize runs on
            # DVE; the last gate tail's Exp goes BEFORE any MLP Sigmoid so
            # the ACT table is switched exactly once, and the chunk-3
            # AllToAll launches as early as possible.
            ps01 = [mlp1_mm(0, mt2) for mt2 in (0, 1)]
            gate_tail(prev_norm[0])
            prev_norm[0] = None
            alltoall(3)
            # round-1 readback: Pool-only (DMAs + selects) so nothing on
            # the busy engines queues behind the collective-done semaphore
            rbA = readback(1, 0, nc.gpsimd)
            rbB = readback(1, 1, nc.gpsimd)
            for k, mt2 in enumerate((0, 1)):
                mlp1_sig(0, mt2, ps01[k])
            mlp1_piece(0, 2, NKL // 2)
            mlp2(0)
            rb_select(1, 0, *rbA, nc.gpsimd)
            rb_select(1, 1, *rbB, nc.gpsimd)
            mlp1_piece(1, 0, NKL // 2)
            mlp2(1)


def _host_prep(x, Wq, Wkv_down, Wk_up, Wv_up, Wgate, bgate, Wo1, Wo2):
    half = DH // 2
    pos = np.arange(T, dtype=np.float32)
    inv_freq = 1.0 / (ROPE_BASE ** (np.arange(half, dtype=np.float32) / half))
    ang = pos[:, None] * inv_freq            # [T, 32]
    cos_tab = np.cos(ang).T.astype(np.float32)   # [32, T]
    sin_tab = np.sin(ang).T.astype(np.float32)
    cs64 = np.concatenate([cos_tab, cos_tab], 0)          # [64, T]
    ns64 = np.concatenate([-sin_tab, sin_tab], 0)         # [64, T]
    bf = lambda a: np.ascontiguousarray(a).astype(ml_dtypes.bfloat16)
    cs2 = bf(np.concatenate([cs64, cs64], 0))             # [128, T]
    ns2 = bf(np.concatenate([ns64, ns64], 0))

    # strict-lower triangle (key r, col c of the 128-wide diag block is
    # INVALID iff c < r); negid @ tri adds NEG_BIG there
    rr = np.arange(128)[:, None]
    cc = np.arange(128)[None, :]
    tri = (cc < rr).astype(np.float32)
    negid = np.diag(np.full(128, NEG_BIG, np.float32))
    ones65 = np.zeros((65, DH), np.float32)
    ones65[64, :] = 1.0
    # rope half-swap permutation: within each 64-row block, rows 0:32 <-> 32:64
    idx = np.arange(128)
    swp = (idx // 64) * 64 + ((idx % 64) + 32) % 64
    perm = np.zeros((128, 128), np.float32)
    perm[swp, idx] = 1.0

    in_maps = []
    for core in range(N_CORES):
        b, g = divmod(core, 4)
        sa = 1.0 - b
        in_maps.append({
            "xT": bf(x[b].T),
            "wq": bf(Wq[:, g * NHL * DH:(g + 1) * NHL * DH]),
            "wkv": bf(Wkv_down),
            "wk": bf(np.concatenate([Wk_up[:, g * DH:(g + 1) * DH]] * 2, 1)),
            "perm": bf(perm),
            "wv": bf(Wv_up[:, g * DH:(g + 1) * DH]),
            "wg": bf(Wgate),
            # tanh-sigmoid bias: tanh(0.5*z + 0.5*b); the paired /2s of the
            # gate and silu sigmoids are folded into Wo1 and Wo2
            "bgn": np.ascontiguousarray(0.5 * bgate[:, None]).astype(
                np.float32),
            "wo1": bf(0.5 * Wo1),
            "wo2": bf(0.5 * Wo2),
            "cs2": cs2,
            "ns2": ns2,
            "tri": bf(tri),
            "negid": bf(negid),
            "ones65": bf(ones65),
            "selA": np.full((128, 1), sa, np.float32),
            "selB": np.full((128, 1), 1.0 - sa, np.float32),
        })
    return in_maps


def kernel(**inputs) -> np.ndarray:
    if "nc" not in _cache:
        _cache["nc"] = _build_nc()
    nc = _cache["nc"]
    in_maps = _host_prep(**inputs)
    res = run_bass_kernel_spmd(nc, in_maps, core_ids=list(range(N_CORES)))
    y = np.empty((B, T, DM), np.float32)
    for core in range(N_CORES):
        b, q = divmod(core, 4)
        yc = res.results[core]["yT"].astype(np.float32).T    # [512, DM]
        for r in range(2):
            for jr in range(2):
                j = 2 * r + jr
                rows = yc[r * RT + jr * 128: r * RT + jr * 128 + 128]
                y[b, 512 * j + 128 * q: 512 * j + 128 * q + 128, :] = rows
    return y


if __name__ == "__main__":
    import reference
    inputs = {k: np.asarray(v) for k, v in reference.setup_inputs().items()}
    out = kernel(**inputs)
    want = np.asarray(reference.reference(**inputs))
    err = np.abs(out - want).max()
    rel = err / np.abs(want).max()
    print(f"max abs err {err:.4e}, rel {rel:.4e}")
